# revision 16
# baseline (speedup 1.0000x reference)
"""Trainium2 Bass kernel for the brute-force antisymmetrized ResNet.

Math (per walker b):
    feats[i,j] = concat(x1[P1[i]], x2[P2[j]]).reshape(24)    (576 = 24*24 perm pairs)
    y0 = tanh(feats @ W0 + b0)
    y1 = tanh(y0 @ W1 + b1) + y0
    y2 = tanh(y1 @ W2 + b2) + y1
    out[b] = log| sum_{i,j} s1[i] s2[j] (y2 @ Wf + bf) |

Strategy:
  - Data-parallel over the 512 walkers: 64 walkers per NeuronCore x 8 cores.
  - First layer is factored: y0pre[b,i,j] = u1[b,i] + u2[b,j] with u1/u2 from
    two tiny matmuls; broadcast-adds build the 576 rows per walker on the DVE.
  - Permutations are sign-sorted into quadrants [(+,+), (-,-), (+,-), (-,+)]
    x 12 x 12 so each walker's first 288 rows have pair-sign +1 and the last
    288 have -1 (enables the pair-difference reduction below).
  - All three matmul stages run as split-2 fp16 limb matmuls: each activation
    x is split as x = hi + lo with hi = fp16(x), lo = fp16(x - hi); weights
    are host-split the same way.  psum += Whi@hi + Whi@lo + Wlo@hi keeps full
    fp32-level accuracy (limb products are exact; fp16 subnormals pass the PE
    unmodified) at 1 cycle/row instead of fp32's 4 -- the PE cost of the two
    256x256 layers drops 4->3 cycles/row.
  - hi limbs come free-ish from ScalarE (tanh/copy activations with fp16
    output dtype = exact RNE); lo limbs are one DVE subtract each.
  - Accuracy-critical reduction: y2 = y1 + tanh2 is materialized in-place
    over the h0 ring, then per group d = y2[+rows] - y2[-rows] (pair
    difference cancels the per-channel mean before any long fp32
    accumulation), then 48-element chunk sums on the Pool engine; the 6
    chunk sums per (channel, walker) are combined on the host in fp64.
    This removes the 288-term sequential-fp32 accumulation error that
    dominated the old scheme (~3e-2 rel err -> ~3e-3).
  - Device returns [128, 768] chunk sums; host applies Wf in fp64 and log|.|.
"""

import itertools

import numpy as np

N1 = 4
N2 = 4
D = 3
BATCH = 512
NDENSE = 256
NCORES = 8
NPERM = 24              # 4!
NPAIR = NPERM * NPERM   # 576
HALFPAIR = NPAIR // 2   # 288 rows of each sign per walker
QUAD = NPERM // 2 * (NPERM // 2)            # 144 rows per quadrant
WALKERS_PER_CORE = BATCH // NCORES          # 64
ROWS_PER_CORE = WALKERS_PER_CORE * NPAIR    # 36864
TILE = 512                                  # matmul moving-dim tile
GROUP_WALKERS = 8                           # walkers per h0-ring group
GROUP_ROWS = GROUP_WALKERS * NPAIR          # 4608 = 9 * TILE
TILES_PER_GROUP = GROUP_ROWS // TILE        # 9
NGROUPS = ROWS_PER_CORE // GROUP_ROWS       # 8
NTILES = NGROUPS * TILES_PER_GROUP          # 72
UCOLS = WALKERS_PER_CORE * NPERM            # 1536 u-columns per core
K1 = N1 * D + 1                             # 13: x1 features + ones row (b0)
K2 = N2 * D                                 # 12
CHUNK = 48                                  # level-1 chunk of the 288 pair-diffs
NCH = HALFPAIR // CHUNK                     # 6 chunks per (channel, walker)
VCOLS = NGROUPS * 2 * GROUP_WALKERS * NCH   # 768 output cols


def _perms_and_signs(n):
    P = np.array(list(itertools.permutations(range(n))), dtype=np.int32)
    triu = np.triu(np.ones((n, n), dtype=np.int64), 1)
    inv = np.sum((P[:, :, None] > P[:, None, :]) * triu, axis=(1, 2))
    signs = np.where(inv % 2 == 0, 1.0, -1.0).astype(np.float32)
    return P, signs


_P1, _S1 = _perms_and_signs(N1)
_P2, _S2 = _perms_and_signs(N2)

# sign-sorted perm orders: 12 even perms then 12 odd perms
_ISORT = np.concatenate([np.where(_S1 > 0)[0], np.where(_S1 < 0)[0]])
_JSORT = np.concatenate([np.where(_S2 > 0)[0], np.where(_S2 < 0)[0]])
# quadrants (a, b): pair-sign = +1 for the first two, -1 for the last two
_QUADS = [(0, 0), (1, 1), (0, 1), (1, 0)]

_cached = {}
_last_results = None  # BassKernelResults of the most recent run (for profiling)


def _build_nc(with_bias: bool):
    """Build + compile the 8-core SPMD Tile kernel (cached)."""
    key = bool(with_bias)
    if key in _cached:
        return _cached[key]

    import concourse.bacc as bacc
    import concourse.tile as tile
    from concourse import mybir

    FP = mybir.dt.float32
    F16 = mybir.dt.float16
    TANH = mybir.ActivationFunctionType.Tanh
    COPY = mybir.ActivationFunctionType.Copy
    AXX = mybir.AxisListType.X

    nc = bacc.Bacc(
        "TRN2",
        target_bir_lowering=False,
        debug=False,
        num_devices=NCORES,
    )

    def din(name, shape, dt=F16):
        return nc.dram_tensor(name, shape, dt, kind="ExternalInput").ap()

    x1fh_d = din("x1fh", [K1, UCOLS])
    x1fl_d = din("x1fl", [K1, UCOLS])
    x2fh_d = din("x2fh", [K2, UCOLS])
    x2fl_d = din("x2fl", [K2, UCOLS])
    x1wh_d = din("x1wh", [K1, NDENSE])
    x1wl_d = din("x1wl", [K1, NDENSE])
    x2wh_d = din("x2wh", [K2, NDENSE])
    x2wl_d = din("x2wl", [K2, NDENSE])
    wd = {}
    for l, half in itertools.product((1, 2), ("a", "b")):
        for part in ("h", "l"):
            wd[f"w{l}{half}{part}"] = din(f"w{l}{half}{part}", [128, NDENSE])
    if with_bias:
        b_d = din("b12", [128, 4], FP)
    v_d = nc.dram_tensor("v", [128, VCOLS], FP, kind="ExternalOutput").ap()

    with tile.TileContext(nc) as tc:
        with (
            tc.tile_pool(name="consts", bufs=1) as cpool,
            tc.tile_pool(name="acts", bufs=3) as apool,
            tc.tile_pool(name="limbs", bufs=3) as lpool,
            tc.tile_pool(name="h0ring", bufs=2) as hpool,
            tc.tile_pool(name="vout", bufs=1) as vpool,
            tc.tile_pool(name="ps", bufs=4, space="PSUM") as pspool,
        ):
            def cload(name, shape, src, dt=F16):
                t = cpool.tile(shape, dt, tag=name)
                nc.sync.dma_start(t[:], src[:])
                return t

            x1fh = cload("x1fh", [K1, UCOLS], x1fh_d)
            x1fl = cload("x1fl", [K1, UCOLS], x1fl_d)
            x2fh = cload("x2fh", [K2, UCOLS], x2fh_d)
            x2fl = cload("x2fl", [K2, UCOLS], x2fl_d)
            x1wh = cload("x1wh", [K1, NDENSE], x1wh_d)
            x1wl = cload("x1wl", [K1, NDENSE], x1wl_d)
            x2wh = cload("x2wh", [K2, NDENSE], x2wh_d)
            x2wl = cload("x2wl", [K2, NDENSE], x2wl_d)
            w = {
                k: cload(k, [128, NDENSE], d) for k, d in wd.items()
            }
            if with_bias:
                bsb = cload("b12", [128, 4], b_d, FP)  # b1h0 b1h1 b2h0 b2h1

            u1s = cpool.tile([128, 2, UCOLS], FP, tag="u1s")
            u2s = cpool.tile([128, 2, UCOLS], FP, tag="u2s")
            vout = vpool.tile([128, VCOLS], FP, tag="v")

            h0tiles = {}

            def h0tile(g):
                if g not in h0tiles:
                    h0tiles[g] = hpool.tile(
                        [128, 2, GROUP_ROWS], FP, tag="h0g", name=f"h0g{g}"
                    )
                return h0tiles[g]

            def brd_walker(g, w):
                """Broadcast-add h0 rows of walker w of group g (4 quads).

                Walker-granular so each piece is emitted ~8 tiles before the
                tanh0 that consumes it, instead of crowding group boundaries.
                """
                h0g = h0tile(g)
                uc = (g * GROUP_WALKERS + w) * NPERM
                for q, (a, b2) in enumerate(_QUADS):
                    base = w * NPAIR + q * QUAD
                    out_ap = h0g[:, :, base:base + QUAD].rearrange(
                        "p h (i j) -> p h i j", j=12
                    )
                    in1 = u1s[:, :, uc + a * 12:uc + (a + 1) * 12].rearrange(
                        "p h (i u) -> p h i u", u=1
                    ).broadcast_to([128, 2, 12, 12])
                    in2 = u2s[:, :, uc + b2 * 12:uc + (b2 + 1) * 12].rearrange(
                        "p h (u j) -> p h u j", u=1
                    ).broadcast_to([128, 2, 12, 12])
                    nc.vector.tensor_add(out_ap, in1, in2)

            t0limbs = {}

            def head(j):
                """tanh0 (in place in the ring) + fp16 limb split for tile j.

                The lo subtract runs on the Pool engine: its result is not
                needed until layer-1 of tile j, a full iteration later, and
                DVE is the tighter budget.
                """
                g, s = divmod(j, TILES_PER_GROUP)
                ap = h0tile(g)[:, :, s * TILE:(s + 1) * TILE]
                nc.scalar.activation(ap, ap, TANH)
                th = lpool.tile([128, 2, TILE], F16, tag="t0hi")
                nc.gpsimd.tensor_copy(th[:], ap)
                tl = lpool.tile([128, 2, TILE], F16, tag="t0lo")
                nc.gpsimd.tensor_sub(tl[:], ap, th[:])
                t0limbs[j] = (th, tl)

            def layer_mms(ps, wl_, hi, lo):
                """6 split-2 limb matmuls per output half into psum ps.

                All hi-consuming products are issued first (both halves
                interleaved) and the four lo-consuming products last, giving
                the lo-limb producer ~8 matmul slots of extra slack before
                the PE needs its output.
                """
                wah, wal, wbh, wbl = wl_
                o = [ps[:, m * TILE:(m + 1) * TILE] for m in (0, 1)]
                msl = [slice(m * 128, (m + 1) * 128) for m in (0, 1)]
                for m in (0, 1):
                    nc.tensor.matmul(o[m], wah[:, msl[m]], hi[:, 0, :],
                                     start=True, stop=False)
                    nc.tensor.matmul(o[m], wbh[:, msl[m]], hi[:, 1, :],
                                     start=False, stop=False)
                for m in (0, 1):
                    nc.tensor.matmul(o[m], wal[:, msl[m]], hi[:, 0, :],
                                     start=False, stop=False)
                    nc.tensor.matmul(o[m], wbl[:, msl[m]], hi[:, 1, :],
                                     start=False, stop=False)
                for m in (0, 1):
                    nc.tensor.matmul(o[m], wah[:, msl[m]], lo[:, 0, :],
                                     start=False, stop=False)
                    nc.tensor.matmul(o[m], wbh[:, msl[m]], lo[:, 1, :],
                                     start=False, stop=True)

            w1limbs = (w["w1ah"], w["w1al"], w["w1bh"], w["w1bl"])
            w2limbs = (w["w2ah"], w["w2al"], w["w2bh"], w["w2bl"])

            # ---- u1s/u2s: first-layer partials, columns (walker, sorted perm)
            for c in range(UCOLS // TILE):
                csl = slice(c * TILE, (c + 1) * TILE)
                for (usb, xfh, xfl, xwh, xwl) in (
                    (u1s, x1fh, x1fl, x1wh, x1wl),
                    (u2s, x2fh, x2fl, x2wh, x2wl),
                ):
                    psu = pspool.tile([128, 2 * TILE], FP, tag="ps")
                    for h in (0, 1):
                        out = psu[:, h * TILE:(h + 1) * TILE]
                        hsl = slice(h * 128, (h + 1) * 128)
                        nc.tensor.matmul(out, xwh[:, hsl], xfh[:, csl],
                                         start=True, stop=False)
                        nc.tensor.matmul(out, xwh[:, hsl], xfl[:, csl],
                                         start=False, stop=False)
                        nc.tensor.matmul(out, xwl[:, hsl], xfh[:, csl],
                                         start=False, stop=True)
                    nc.scalar.activation(
                        usb[:, :, csl],
                        psu[:].rearrange("p (h r) -> p h r", h=2),
                        COPY,
                    )
                if c == 0:
                    # tile 0 covers only walker 0's rows: start its tanh0 and
                    # layer-1 ASAP, then fill in the rest of group 0
                    brd_walker(0, 0)
                    head(0)
                    brd_walker(0, 1)

            pending_reduce = []

            def walker_diff(g, w):
                """d = y2[+rows] - y2[-rows] for walker w, in place (Pool)."""
                h0g = h0tiles[g]
                base = w * NPAIR
                plus = h0g[:, :, base:base + HALFPAIR]
                minus = h0g[:, :, base + HALFPAIR:base + NPAIR]
                nc.gpsimd.tensor_sub(plus, plus, minus)
                pending_reduce.append((g, w))

            def walker_reduce(g, w):
                """48-chunk sums of walker w's pair-differences -> vout (DVE).

                Chunks are combined on the host in fp64; runs one tile after
                walker_diff so the DVE never waits on the Pool sub.
                """
                h0g = h0tiles[g]
                base = w * NPAIR
                plus = h0g[:, :, base:base + HALFPAIR]
                inp = plus.rearrange("p h (c e) -> p h c e", e=CHUNK)
                c0 = (g * GROUP_WALKERS + w) * 2 * NCH
                out = vout[:, c0:c0 + 2 * NCH].rearrange(
                    "p (h c) -> p h c", c=NCH
                )
                nc.vector.reduce_sum(out, inp, axis=AXX)

            def reduce_flush():
                while pending_reduce:
                    walker_reduce(*pending_reduce.pop(0))

            def mid_a(j, ps1):
                """tanh1 + residual for tile j."""
                g, s = divmod(j, TILES_PER_GROUP)
                sl3 = h0tiles[g][:, :, s * TILE:(s + 1) * TILE]
                t1 = apool.tile([128, 2, TILE], FP, tag="t1")
                if with_bias:
                    for m in (0, 1):
                        nc.scalar.activation(
                            t1[:, m, :], ps1[:, m * TILE:(m + 1) * TILE],
                            TANH, bias=bsb[:, m:m + 1],
                        )
                else:
                    nc.scalar.activation(
                        t1[:], ps1[:].rearrange("p (h r) -> p h r", h=2), TANH
                    )
                # residual 1, in place: t1 <- t1 + tanh0
                nc.vector.tensor_add(t1[:], t1[:], sl3)
                return t1

            def mid_b(t1):
                """fp16 limb split of t1."""
                t1h = lpool.tile([128, 2, TILE], F16, tag="t1hi")
                nc.scalar.activation(t1h[:], t1[:], COPY)
                t1l = lpool.tile([128, 2, TILE], F16, tag="t1lo")
                nc.vector.tensor_sub(t1l[:], t1[:], t1h[:])
                return t1h, t1l

            def fin(j, t1, ps2):
                """tanh2 + y2 into the ring + completed-walker reductions."""
                g, s = divmod(j, TILES_PER_GROUP)
                sl3 = h0tiles[g][:, :, s * TILE:(s + 1) * TILE]
                # chunk-reduce (DVE) for the walker diffed last tile: its
                # Pool subtract finished during the previous iteration
                reduce_flush()
                t2 = apool.tile([128, 2, TILE], FP, tag="t2")
                if with_bias:
                    for m in (0, 1):
                        nc.scalar.activation(
                            t2[:, m, :], ps2[:, m * TILE:(m + 1) * TILE],
                            TANH, bias=bsb[:, 2 + m:3 + m],
                        )
                else:
                    nc.scalar.activation(
                        t2[:], ps2[:].rearrange("p (h r) -> p h r", h=2), TANH
                    )
                # y2 = t1 + tanh2, overwriting this tile's rows in the ring
                # (two tiles per group ride the Pool to keep DVE off the
                # critical path)
                if s in (2, 6):
                    nc.gpsimd.tensor_add(sl3, t1[:], t2[:])
                else:
                    nc.vector.tensor_add(sl3, t1[:], t2[:])
                # pair-difference for walkers whose y2 rows completed a tile
                # ago (keeps the Pool reads clear of in-flight y2 adds)
                if s == 0 and g >= 1:
                    walker_diff(g - 1, GROUP_WALKERS - 1)
                if s >= 2:
                    walker_diff(g, s - 2)

            # Software pipeline, 4 stages deep so no engine ever waits on the
            # cross-engine act/limb chains: iteration j runs L1(j) and
            # L2(j-2) back-to-back on the PE (exactly 8 PSUM banks), with
            # tanh1/limbs of j-1 and tanh2/y2/reductions of j-3 in flight.
            ps1reg = {}
            midreg = {}
            for j in range(NTILES + 3):
                g, s = divmod(j, TILES_PER_GROUP)

                # next group's h0 rows for walker s: emitted 8 tiles before
                # its tanh0 and 3+ tiles after the grandparent group's reads
                # of the same ring region (write-after-read safe)
                if s < GROUP_WALKERS and g + 1 < NGROUPS and j < NTILES:
                    brd_walker(g + 1, s)
                if g == 0 and s < GROUP_WALKERS - 2:
                    brd_walker(0, s + 2)

                if j - 1 >= 0 and j - 1 < NTILES:
                    t1 = mid_a(j - 1, ps1reg.pop(j - 1))
                    midreg[j - 1] = t1

                if j < NTILES:
                    ps1 = pspool.tile([128, 2 * TILE], FP, tag="ps")
                    th, tl = t0limbs.pop(j)
                    layer_mms(ps1, w1limbs, th, tl)
                    ps1reg[j] = ps1
                    if j + 1 < NTILES:
                        head(j + 1)

                if j - 1 >= 0 and j - 1 < NTILES:
                    midreg[j - 1] = (
                        midreg[j - 1],
                        *mid_b(midreg[j - 1]),
                    )

                if j - 2 >= 0 and j - 2 < NTILES:
                    t1_, t1h, t1l = midreg[j - 2]
                    ps2 = pspool.tile([128, 2 * TILE], FP, tag="ps")
                    layer_mms(ps2, w2limbs, t1h, t1l)
                    midreg[j - 2] = (t1_, ps2)

                if j - 3 >= 0:
                    t1_, ps2 = midreg.pop(j - 3)
                    fin(j - 3, t1_, ps2)

                # drain the first half of vout early (groups 0-3 reduced)
                if j == NTILES // 2 + 5:
                    nc.sync.dma_start(
                        v_d[:, 0:VCOLS // 2], vout[:, 0:VCOLS // 2]
                    )

            walker_diff(NGROUPS - 1, GROUP_WALKERS - 1)
            reduce_flush()
            nc.sync.dma_start(v_d[:, VCOLS // 2:], vout[:, VCOLS // 2:])

    nc.compile()
    _cached[key] = nc
    return nc


def _build_feats(x1, x2):
    """Per-walker first-layer inputs in sign-sorted perm order.

    Returns (X1f [B, 24, 13], X2f [B, 24, 12]): X1f[b, ip] = flattened
    x1[b, P1[_ISORT[ip]]] + trailing 1.0 (carries b0); X2f likewise, no ones.
    """
    B = x1.shape[0]
    xp1 = x1[:, _P1[_ISORT], :].reshape(B, NPERM, N1 * D)
    xp2 = x2[:, _P2[_JSORT], :].reshape(B, NPERM, N2 * D)
    X1f = np.empty((B, NPERM, K1), dtype=np.float32)
    X1f[:, :, :N1 * D] = xp1
    X1f[:, :, N1 * D] = 1.0
    return X1f, np.ascontiguousarray(xp2)


def _split16(a):
    """fp16 split-2 limbs of an fp32 array."""
    hi = a.astype(np.float16)
    lo = (a.astype(np.float32) - hi.astype(np.float32)).astype(np.float16)
    return hi, lo


def _make_in_maps(x1, x2, W0, b0, W1, b1, W2, b2):
    with_bias = bool(np.any(b1) or np.any(b2))
    X1f, X2f = _build_feats(x1, x2)
    x1w = np.ascontiguousarray(
        np.concatenate([W0[:N1 * D], b0[None, :]], axis=0)
    )  # [13, 256]
    x2w = np.ascontiguousarray(W0[N1 * D:])  # [12, 256]
    x1wh, x1wl = _split16(x1w)
    x2wh, x2wl = _split16(x2w)
    wl = {}
    for l, W in ((1, W1), (2, W2)):
        for half, sl in (("a", slice(0, 128)), ("b", slice(128, 256))):
            h, lo = _split16(np.ascontiguousarray(W[sl]))
            wl[f"w{l}{half}h"] = h
            wl[f"w{l}{half}l"] = lo
    in_maps = []
    for c in range(NCORES):
        sl = slice(c * WALKERS_PER_CORE, (c + 1) * WALKERS_PER_CORE)
        x1fh, x1fl = _split16(
            np.ascontiguousarray(X1f[sl].reshape(UCOLS, K1).T)
        )
        x2fh, x2fl = _split16(
            np.ascontiguousarray(X2f[sl].reshape(UCOLS, K2).T)
        )
        m = {
            "x1fh": x1fh, "x1fl": x1fl, "x2fh": x2fh, "x2fl": x2fl,
            "x1wh": x1wh, "x1wl": x1wl, "x2wh": x2wh, "x2wl": x2wl,
            **wl,
        }
        if with_bias:
            bm = np.zeros((128, 4), dtype=np.float32)
            bm[:, 0] = b1[0:128]
            bm[:, 1] = b1[128:256]
            bm[:, 2] = b2[0:128]
            bm[:, 3] = b2[128:256]
            m["b12"] = bm
        in_maps.append(m)
    return with_bias, in_maps


def _combine_core(v):
    """vout [128, 768] -> per-walker signed sums u [256, 64] in fp64.

    vout layout: [w(64), h(2), chunk(6)]; u[h*128+p, w] = sum_chunks (the
    pair-difference already applied the pair signs).
    """
    acc = v.reshape(128, WALKERS_PER_CORE, 2, NCH).astype(np.float64).sum(
        axis=-1
    )
    return np.transpose(acc, (2, 0, 1)).reshape(2 * 128, WALKERS_PER_CORE)


def _finish(v_per_core, Wf, bf):
    """per-core vout -> log|anti| [BATCH]."""
    out = np.empty((BATCH,), dtype=np.float32)
    wf64 = Wf[:, 0].astype(np.float64)
    # sum of pair signs is exactly 0, so bf drops out of the signed sum
    for c in range(NCORES):
        u = _combine_core(v_per_core[c])
        anti = wf64 @ u
        out[c * WALKERS_PER_CORE:(c + 1) * WALKERS_PER_CORE] = np.log(
            np.abs(anti)
        ).astype(np.float32)
    return out


def kernel(x1, x2, W0, b0, W1, b1, W2, b2, Wf, bf):
    from concourse.bass_utils import run_bass_kernel_spmd

    x1 = np.asarray(x1, dtype=np.float32)
    x2 = np.asarray(x2, dtype=np.float32)
    W0 = np.asarray(W0, dtype=np.float32)
    b0 = np.asarray(b0, dtype=np.float32)
    W1 = np.asarray(W1, dtype=np.float32)
    b1 = np.asarray(b1, dtype=np.float32)
    W2 = np.asarray(W2, dtype=np.float32)
    b2 = np.asarray(b2, dtype=np.float32)
    Wf = np.asarray(Wf, dtype=np.float32)
    bf = np.asarray(bf, dtype=np.float32)

    with_bias, in_maps = _make_in_maps(x1, x2, W0, b0, W1, b1, W2, b2)
    nc = _build_nc(with_bias)

    res = run_bass_kernel_spmd(nc, in_maps, list(range(NCORES)))
    global _last_results
    _last_results = res

    return _finish([res.results[c]["v"] for c in range(NCORES)], Wf, bf)


# revision 17
# speedup vs baseline: 1.0575x; 1.0575x over previous
"""Trainium2 Bass kernel for the brute-force antisymmetrized ResNet.

Math (per walker b):
    feats[i,j] = concat(x1[P1[i]], x2[P2[j]]).reshape(24)    (576 = 24*24 perm pairs)
    y0 = tanh(feats @ W0 + b0)
    y1 = tanh(y0 @ W1 + b1) + y0
    y2 = tanh(y1 @ W2 + b2) + y1
    out[b] = log| sum_{i,j} s1[i] s2[j] (y2 @ Wf + bf) |

Strategy:
  - Data-parallel over the 512 walkers: 64 walkers per NeuronCore x 8 cores.
  - First layer is factored: y0pre[b,i,j] = u1[b,i] + u2[b,j] with u1/u2 from
    two tiny matmuls; broadcast-adds build the 576 rows per walker on the DVE.
  - Permutations are sign-sorted into quadrants [(+,+), (-,-), (+,-), (-,+)]
    x 12 x 12 so each walker's first 288 rows have pair-sign +1 and the last
    288 have -1 (enables the pair-difference reduction below).
  - All three matmul stages run as split-2 fp16 limb matmuls: each activation
    x is split as x = hi + lo with hi = fp16(x), lo = fp16(x - hi); weights
    are host-split the same way.  psum += Whi@hi + Whi@lo + Wlo@hi keeps full
    fp32-level accuracy (limb products are exact; fp16 subnormals pass the PE
    unmodified) at 1 cycle/row instead of fp32's 4 -- the PE cost of the two
    256x256 layers drops 4->3 cycles/row.
  - hi limbs come free-ish from ScalarE (tanh/copy activations with fp16
    output dtype = exact RNE); lo limbs are one DVE subtract each.
  - Accuracy-critical reduction: y2 = y1 + tanh2 is materialized in-place
    over the h0 ring, then per group d = y2[+rows] - y2[-rows] (pair
    difference cancels the per-channel mean before any long fp32
    accumulation), then 48-element chunk sums on the Pool engine; the 6
    chunk sums per (channel, walker) are combined on the host in fp64.
    This removes the 288-term sequential-fp32 accumulation error that
    dominated the old scheme (~3e-2 rel err -> ~3e-3).
  - Device returns [128, 768] chunk sums; host applies Wf in fp64 and log|.|.
"""

import itertools

import numpy as np

N1 = 4
N2 = 4
D = 3
BATCH = 512
NDENSE = 256
NCORES = 8
NPERM = 24              # 4!
NPAIR = NPERM * NPERM   # 576
HALFPAIR = NPAIR // 2   # 288 rows of each sign per walker
QUAD = NPERM // 2 * (NPERM // 2)            # 144 rows per quadrant
WALKERS_PER_CORE = BATCH // NCORES          # 64
ROWS_PER_CORE = WALKERS_PER_CORE * NPAIR    # 36864
TILE = 512                                  # matmul moving-dim tile
GROUP_WALKERS = 8                           # walkers per h0-ring group
GROUP_ROWS = GROUP_WALKERS * NPAIR          # 4608 = 9 * TILE
TILES_PER_GROUP = GROUP_ROWS // TILE        # 9
NGROUPS = ROWS_PER_CORE // GROUP_ROWS       # 8
NTILES = NGROUPS * TILES_PER_GROUP          # 72
UCOLS = WALKERS_PER_CORE * NPERM            # 1536 u-columns per core
K1 = N1 * D + 1                             # 13: x1 features + ones row (b0)
K2 = N2 * D                                 # 12
CHUNK = 48                                  # level-1 chunk of the 288 pair-diffs
NCH = HALFPAIR // CHUNK                     # 6 chunks per (channel, walker)
VCOLS = NGROUPS * 2 * GROUP_WALKERS * NCH   # 768 output cols


def _perms_and_signs(n):
    P = np.array(list(itertools.permutations(range(n))), dtype=np.int32)
    triu = np.triu(np.ones((n, n), dtype=np.int64), 1)
    inv = np.sum((P[:, :, None] > P[:, None, :]) * triu, axis=(1, 2))
    signs = np.where(inv % 2 == 0, 1.0, -1.0).astype(np.float32)
    return P, signs


_P1, _S1 = _perms_and_signs(N1)
_P2, _S2 = _perms_and_signs(N2)

# sign-sorted perm orders: 12 even perms then 12 odd perms
_ISORT = np.concatenate([np.where(_S1 > 0)[0], np.where(_S1 < 0)[0]])
_JSORT = np.concatenate([np.where(_S2 > 0)[0], np.where(_S2 < 0)[0]])
# quadrants (a, b): pair-sign = +1 for the first two, -1 for the last two
_QUADS = [(0, 0), (1, 1), (0, 1), (1, 0)]

_cached = {}
_last_results = None  # BassKernelResults of the most recent run (for profiling)


def _build_nc(with_bias: bool):
    """Build + compile the 8-core SPMD Tile kernel (cached)."""
    key = bool(with_bias)
    if key in _cached:
        return _cached[key]

    import concourse.bacc as bacc
    import concourse.tile as tile
    from concourse import mybir

    FP = mybir.dt.float32
    F16 = mybir.dt.float16
    TANH = mybir.ActivationFunctionType.Tanh
    COPY = mybir.ActivationFunctionType.Copy
    AXX = mybir.AxisListType.X

    nc = bacc.Bacc(
        "TRN2",
        target_bir_lowering=False,
        debug=False,
        num_devices=NCORES,
    )

    def din(name, shape, dt=F16):
        return nc.dram_tensor(name, shape, dt, kind="ExternalInput").ap()

    x1fh_d = din("x1fh", [K1, UCOLS])
    x1fl_d = din("x1fl", [K1, UCOLS])
    x2fh_d = din("x2fh", [K2, UCOLS])
    x2fl_d = din("x2fl", [K2, UCOLS])
    x1wh_d = din("x1wh", [K1, NDENSE])
    x1wl_d = din("x1wl", [K1, NDENSE])
    x2wh_d = din("x2wh", [K2, NDENSE])
    x2wl_d = din("x2wl", [K2, NDENSE])
    wd = {}
    for l, half in itertools.product((1, 2), ("a", "b")):
        for part in ("h", "l"):
            wd[f"w{l}{half}{part}"] = din(f"w{l}{half}{part}", [128, NDENSE])
    if with_bias:
        b_d = din("b12", [128, 4], FP)
    v_d = nc.dram_tensor("v", [128, VCOLS], FP, kind="ExternalOutput").ap()

    with tile.TileContext(nc) as tc:
        with (
            tc.tile_pool(name="consts", bufs=1) as cpool,
            tc.tile_pool(name="acts", bufs=3) as apool,
            tc.tile_pool(name="limbs", bufs=3) as lpool,
            tc.tile_pool(name="h0ring", bufs=2) as hpool,
            tc.tile_pool(name="vout", bufs=1) as vpool,
            tc.tile_pool(name="ps", bufs=4, space="PSUM") as pspool,
        ):
            def cload(name, shape, src, dt=F16):
                t = cpool.tile(shape, dt, tag=name)
                nc.sync.dma_start(t[:], src[:])
                return t

            x1fh = cload("x1fh", [K1, UCOLS], x1fh_d)
            x1fl = cload("x1fl", [K1, UCOLS], x1fl_d)
            x2fh = cload("x2fh", [K2, UCOLS], x2fh_d)
            x2fl = cload("x2fl", [K2, UCOLS], x2fl_d)
            x1wh = cload("x1wh", [K1, NDENSE], x1wh_d)
            x1wl = cload("x1wl", [K1, NDENSE], x1wl_d)
            x2wh = cload("x2wh", [K2, NDENSE], x2wh_d)
            x2wl = cload("x2wl", [K2, NDENSE], x2wl_d)
            w = {
                k: cload(k, [128, NDENSE], d) for k, d in wd.items()
            }
            if with_bias:
                bsb = cload("b12", [128, 4], b_d, FP)  # b1h0 b1h1 b2h0 b2h1

            u1s = cpool.tile([128, 2, UCOLS], FP, tag="u1s")
            u2s = cpool.tile([128, 2, UCOLS], FP, tag="u2s")
            vout = vpool.tile([128, VCOLS], FP, tag="v")

            h0tiles = {}

            def h0tile(g):
                if g not in h0tiles:
                    h0tiles[g] = hpool.tile(
                        [128, 2, GROUP_ROWS], FP, tag="h0g", name=f"h0g{g}"
                    )
                return h0tiles[g]

            def brd_walker(g, w):
                """Broadcast-add h0 rows of walker w of group g (4 quads).

                Walker-granular so each piece is emitted ~8 tiles before the
                tanh0 that consumes it, instead of crowding group boundaries.
                """
                h0g = h0tile(g)
                uc = (g * GROUP_WALKERS + w) * NPERM
                for q, (a, b2) in enumerate(_QUADS):
                    base = w * NPAIR + q * QUAD
                    out_ap = h0g[:, :, base:base + QUAD].rearrange(
                        "p h (i j) -> p h i j", j=12
                    )
                    in1 = u1s[:, :, uc + a * 12:uc + (a + 1) * 12].rearrange(
                        "p h (i u) -> p h i u", u=1
                    ).broadcast_to([128, 2, 12, 12])
                    in2 = u2s[:, :, uc + b2 * 12:uc + (b2 + 1) * 12].rearrange(
                        "p h (u j) -> p h u j", u=1
                    ).broadcast_to([128, 2, 12, 12])
                    nc.vector.tensor_add(out_ap, in1, in2)

            t0limbs = {}

            def head(j):
                """tanh0 (in place in the ring) + fp16 limb split for tile j.

                The lo subtract runs on the Pool engine: its result is not
                needed until layer-1 of tile j, a full iteration later, and
                DVE is the tighter budget.
                """
                g, s = divmod(j, TILES_PER_GROUP)
                ap = h0tile(g)[:, :, s * TILE:(s + 1) * TILE]
                nc.scalar.activation(ap, ap, TANH)
                th = lpool.tile([128, 2, TILE], F16, tag="t0hi")
                nc.gpsimd.tensor_copy(th[:], ap)
                tl = lpool.tile([128, 2, TILE], F16, tag="t0lo")
                nc.gpsimd.tensor_sub(tl[:], ap, th[:])
                t0limbs[j] = (th, tl)

            def layer_mms(ps, wl_, hi, lo):
                """6 split-2 limb matmuls per output half into psum ps.

                All hi-consuming products are issued first (both halves
                interleaved) and the four lo-consuming products last, giving
                the lo-limb producer ~8 matmul slots of extra slack before
                the PE needs its output.
                """
                wah, wal, wbh, wbl = wl_
                o = [ps[:, m * TILE:(m + 1) * TILE] for m in (0, 1)]
                msl = [slice(m * 128, (m + 1) * 128) for m in (0, 1)]
                for m in (0, 1):
                    nc.tensor.matmul(o[m], wah[:, msl[m]], hi[:, 0, :],
                                     start=True, stop=False)
                    nc.tensor.matmul(o[m], wbh[:, msl[m]], hi[:, 1, :],
                                     start=False, stop=False)
                for m in (0, 1):
                    nc.tensor.matmul(o[m], wal[:, msl[m]], hi[:, 0, :],
                                     start=False, stop=False)
                    nc.tensor.matmul(o[m], wbl[:, msl[m]], hi[:, 1, :],
                                     start=False, stop=False)
                for m in (0, 1):
                    nc.tensor.matmul(o[m], wah[:, msl[m]], lo[:, 0, :],
                                     start=False, stop=False)
                    nc.tensor.matmul(o[m], wbh[:, msl[m]], lo[:, 1, :],
                                     start=False, stop=True)

            w1limbs = (w["w1ah"], w["w1al"], w["w1bh"], w["w1bl"])
            w2limbs = (w["w2ah"], w["w2al"], w["w2bh"], w["w2bl"])

            # ---- u1s/u2s: first-layer partials, columns (walker, sorted perm)
            for c in range(UCOLS // TILE):
                csl = slice(c * TILE, (c + 1) * TILE)
                for (usb, xfh, xfl, xwh, xwl) in (
                    (u1s, x1fh, x1fl, x1wh, x1wl),
                    (u2s, x2fh, x2fl, x2wh, x2wl),
                ):
                    psu = pspool.tile([128, 2 * TILE], FP, tag="ps")
                    for h in (0, 1):
                        out = psu[:, h * TILE:(h + 1) * TILE]
                        hsl = slice(h * 128, (h + 1) * 128)
                        nc.tensor.matmul(out, xwh[:, hsl], xfh[:, csl],
                                         start=True, stop=False)
                        nc.tensor.matmul(out, xwh[:, hsl], xfl[:, csl],
                                         start=False, stop=False)
                        nc.tensor.matmul(out, xwl[:, hsl], xfh[:, csl],
                                         start=False, stop=True)
                    nc.scalar.activation(
                        usb[:, :, csl],
                        psu[:].rearrange("p (h r) -> p h r", h=2),
                        COPY,
                    )
                if c == 0:
                    # tile 0 covers only walker 0's rows: start its tanh0 and
                    # layer-1 ASAP, then fill in the rest of group 0
                    brd_walker(0, 0)
                    head(0)
                    brd_walker(0, 1)

            pending_reduce = []

            def walker_diff(g, w):
                """d = y2[+rows] - y2[-rows] for walker w, in place (Pool)."""
                h0g = h0tiles[g]
                base = w * NPAIR
                plus = h0g[:, :, base:base + HALFPAIR]
                minus = h0g[:, :, base + HALFPAIR:base + NPAIR]
                nc.gpsimd.tensor_sub(plus, plus, minus)
                pending_reduce.append((g, w))

            def walker_reduce(g, w):
                """48-chunk sums of walker w's pair-differences -> vout (DVE).

                Chunks are combined on the host in fp64; runs one tile after
                walker_diff so the DVE never waits on the Pool sub.
                """
                h0g = h0tiles[g]
                base = w * NPAIR
                plus = h0g[:, :, base:base + HALFPAIR]
                inp = plus.rearrange("p h (c e) -> p h c e", e=CHUNK)
                c0 = (g * GROUP_WALKERS + w) * 2 * NCH
                out = vout[:, c0:c0 + 2 * NCH].rearrange(
                    "p (h c) -> p h c", c=NCH
                )
                nc.vector.reduce_sum(out, inp, axis=AXX)

            def reduce_flush():
                while pending_reduce:
                    walker_reduce(*pending_reduce.pop(0))

            def mid_a(j, ps1):
                """tanh1 + residual for tile j."""
                g, s = divmod(j, TILES_PER_GROUP)
                sl3 = h0tiles[g][:, :, s * TILE:(s + 1) * TILE]
                t1 = apool.tile([128, 2, TILE], FP, tag="t1")
                if with_bias:
                    for m in (0, 1):
                        nc.scalar.activation(
                            t1[:, m, :], ps1[:, m * TILE:(m + 1) * TILE],
                            TANH, bias=bsb[:, m:m + 1],
                        )
                else:
                    nc.scalar.activation(
                        t1[:], ps1[:].rearrange("p (h r) -> p h r", h=2), TANH
                    )
                # residual 1, in place: t1 <- t1 + tanh0
                nc.vector.tensor_add(t1[:], t1[:], sl3)
                return t1

            def mid_b(t1):
                """fp16 limb split of t1."""
                t1h = lpool.tile([128, 2, TILE], F16, tag="t1hi")
                nc.scalar.activation(t1h[:], t1[:], COPY)
                t1l = lpool.tile([128, 2, TILE], F16, tag="t1lo")
                nc.vector.tensor_sub(t1l[:], t1[:], t1h[:])
                return t1h, t1l

            def fin(j, t1, ps2):
                """tanh2 + y2 into the ring + completed-walker reductions."""
                g, s = divmod(j, TILES_PER_GROUP)
                sl3 = h0tiles[g][:, :, s * TILE:(s + 1) * TILE]
                # chunk-reduce (DVE) for the walker diffed last tile: its
                # Pool subtract finished during the previous iteration
                reduce_flush()
                t2 = apool.tile([128, 2, TILE], FP, tag="t2")
                if with_bias:
                    for m in (0, 1):
                        nc.scalar.activation(
                            t2[:, m, :], ps2[:, m * TILE:(m + 1) * TILE],
                            TANH, bias=bsb[:, 2 + m:3 + m],
                        )
                else:
                    nc.scalar.activation(
                        t2[:], ps2[:].rearrange("p (h r) -> p h r", h=2), TANH
                    )
                # y2 = t1 + tanh2, overwriting this tile's rows in the ring
                nc.vector.tensor_add(sl3, t1[:], t2[:])
                # pair-difference for walkers whose y2 rows completed a tile
                # ago (keeps the Pool reads clear of in-flight y2 adds)
                if s == 0 and g >= 1:
                    walker_diff(g - 1, GROUP_WALKERS - 1)
                if s >= 2:
                    walker_diff(g, s - 2)

            # Software pipeline, 4 stages deep so no engine ever waits on the
            # cross-engine act/limb chains: iteration j runs L1(j) and
            # L2(j-2) back-to-back on the PE (exactly 8 PSUM banks), with
            # tanh1/limbs of j-1 and tanh2/y2/reductions of j-3 in flight.
            ps1reg = {}
            midreg = {}
            for j in range(NTILES + 3):
                g, s = divmod(j, TILES_PER_GROUP)

                # next group's h0 rows for walker s: emitted 8 tiles before
                # its tanh0 and 3+ tiles after the grandparent group's reads
                # of the same ring region (write-after-read safe)
                if s < GROUP_WALKERS and g + 1 < NGROUPS and j < NTILES:
                    brd_walker(g + 1, s)
                if g == 0 and s < GROUP_WALKERS - 2:
                    brd_walker(0, s + 2)

                if j - 1 >= 0 and j - 1 < NTILES:
                    t1 = mid_a(j - 1, ps1reg.pop(j - 1))
                    midreg[j - 1] = t1

                if j < NTILES:
                    ps1 = pspool.tile([128, 2 * TILE], FP, tag="ps")
                    th, tl = t0limbs.pop(j)
                    layer_mms(ps1, w1limbs, th, tl)
                    ps1reg[j] = ps1
                    if j + 1 < NTILES:
                        head(j + 1)

                if j - 1 >= 0 and j - 1 < NTILES:
                    midreg[j - 1] = (
                        midreg[j - 1],
                        *mid_b(midreg[j - 1]),
                    )

                if j - 2 >= 0 and j - 2 < NTILES:
                    t1_, t1h, t1l = midreg[j - 2]
                    ps2 = pspool.tile([128, 2 * TILE], FP, tag="ps")
                    layer_mms(ps2, w2limbs, t1h, t1l)
                    midreg[j - 2] = (t1_, ps2)

                if j - 3 >= 0:
                    t1_, ps2 = midreg.pop(j - 3)
                    fin(j - 3, t1_, ps2)

                # drain the first half of vout early (groups 0-3 reduced)
                if j == NTILES // 2 + 5:
                    nc.sync.dma_start(
                        v_d[:, 0:VCOLS // 2], vout[:, 0:VCOLS // 2]
                    )

            walker_diff(NGROUPS - 1, GROUP_WALKERS - 1)
            reduce_flush()
            nc.sync.dma_start(v_d[:, VCOLS // 2:], vout[:, VCOLS // 2:])

    nc.compile()
    _cached[key] = nc
    return nc


def _build_feats(x1, x2):
    """Per-walker first-layer inputs in sign-sorted perm order.

    Returns (X1f [B, 24, 13], X2f [B, 24, 12]): X1f[b, ip] = flattened
    x1[b, P1[_ISORT[ip]]] + trailing 1.0 (carries b0); X2f likewise, no ones.
    """
    B = x1.shape[0]
    xp1 = x1[:, _P1[_ISORT], :].reshape(B, NPERM, N1 * D)
    xp2 = x2[:, _P2[_JSORT], :].reshape(B, NPERM, N2 * D)
    X1f = np.empty((B, NPERM, K1), dtype=np.float32)
    X1f[:, :, :N1 * D] = xp1
    X1f[:, :, N1 * D] = 1.0
    return X1f, np.ascontiguousarray(xp2)


def _split16(a):
    """fp16 split-2 limbs of an fp32 array."""
    hi = a.astype(np.float16)
    lo = (a.astype(np.float32) - hi.astype(np.float32)).astype(np.float16)
    return hi, lo


def _make_in_maps(x1, x2, W0, b0, W1, b1, W2, b2):
    with_bias = bool(np.any(b1) or np.any(b2))
    X1f, X2f = _build_feats(x1, x2)
    x1w = np.ascontiguousarray(
        np.concatenate([W0[:N1 * D], b0[None, :]], axis=0)
    )  # [13, 256]
    x2w = np.ascontiguousarray(W0[N1 * D:])  # [12, 256]
    x1wh, x1wl = _split16(x1w)
    x2wh, x2wl = _split16(x2w)
    wl = {}
    for l, W in ((1, W1), (2, W2)):
        for half, sl in (("a", slice(0, 128)), ("b", slice(128, 256))):
            h, lo = _split16(np.ascontiguousarray(W[sl]))
            wl[f"w{l}{half}h"] = h
            wl[f"w{l}{half}l"] = lo
    in_maps = []
    for c in range(NCORES):
        sl = slice(c * WALKERS_PER_CORE, (c + 1) * WALKERS_PER_CORE)
        x1fh, x1fl = _split16(
            np.ascontiguousarray(X1f[sl].reshape(UCOLS, K1).T)
        )
        x2fh, x2fl = _split16(
            np.ascontiguousarray(X2f[sl].reshape(UCOLS, K2).T)
        )
        m = {
            "x1fh": x1fh, "x1fl": x1fl, "x2fh": x2fh, "x2fl": x2fl,
            "x1wh": x1wh, "x1wl": x1wl, "x2wh": x2wh, "x2wl": x2wl,
            **wl,
        }
        if with_bias:
            bm = np.zeros((128, 4), dtype=np.float32)
            bm[:, 0] = b1[0:128]
            bm[:, 1] = b1[128:256]
            bm[:, 2] = b2[0:128]
            bm[:, 3] = b2[128:256]
            m["b12"] = bm
        in_maps.append(m)
    return with_bias, in_maps


def _combine_core(v):
    """vout [128, 768] -> per-walker signed sums u [256, 64] in fp64.

    vout layout: [w(64), h(2), chunk(6)]; u[h*128+p, w] = sum_chunks (the
    pair-difference already applied the pair signs).
    """
    acc = v.reshape(128, WALKERS_PER_CORE, 2, NCH).astype(np.float64).sum(
        axis=-1
    )
    return np.transpose(acc, (2, 0, 1)).reshape(2 * 128, WALKERS_PER_CORE)


def _finish(v_per_core, Wf, bf):
    """per-core vout -> log|anti| [BATCH]."""
    out = np.empty((BATCH,), dtype=np.float32)
    wf64 = Wf[:, 0].astype(np.float64)
    # sum of pair signs is exactly 0, so bf drops out of the signed sum
    for c in range(NCORES):
        u = _combine_core(v_per_core[c])
        anti = wf64 @ u
        out[c * WALKERS_PER_CORE:(c + 1) * WALKERS_PER_CORE] = np.log(
            np.abs(anti)
        ).astype(np.float32)
    return out


def kernel(x1, x2, W0, b0, W1, b1, W2, b2, Wf, bf):
    from concourse.bass_utils import run_bass_kernel_spmd

    x1 = np.asarray(x1, dtype=np.float32)
    x2 = np.asarray(x2, dtype=np.float32)
    W0 = np.asarray(W0, dtype=np.float32)
    b0 = np.asarray(b0, dtype=np.float32)
    W1 = np.asarray(W1, dtype=np.float32)
    b1 = np.asarray(b1, dtype=np.float32)
    W2 = np.asarray(W2, dtype=np.float32)
    b2 = np.asarray(b2, dtype=np.float32)
    Wf = np.asarray(Wf, dtype=np.float32)
    bf = np.asarray(bf, dtype=np.float32)

    with_bias, in_maps = _make_in_maps(x1, x2, W0, b0, W1, b1, W2, b2)
    nc = _build_nc(with_bias)

    res = run_bass_kernel_spmd(nc, in_maps, list(range(NCORES)))
    global _last_results
    _last_results = res

    return _finish([res.results[c]["v"] for c in range(NCORES)], Wf, bf)
